# revision 19
# baseline (speedup 1.0000x reference)
"""AffineToDenseShift Trainium2 kernel.

Computes out[b,d,h,w,i] = ((A_b - I) @ mesh(d,h,w) + t_b)[i] for the
centered ij meshgrid of shape (160, 192, 224), batch 4, f32.

The field is additively separable: out = f_i(d) + g_i(h) + k_i(w) with
f_i(d) = M[i,0]*(d-cD) + t[i], g_i(h) = M[i,1]*(h-cH), k_i(w) = M[i,2]*(w-cW),
M = A - I.  Inputs are tiny (48 floats/batch); the problem is purely about
materializing and writing the 330 MB output at HBM line rate (~358 GB/s
per NeuronCore -> 115.3 us floor for the 41.3 MB per-core slice).

Sharding: 8 cores = 4 batches x 2 halves of D.  Each core writes a flat
contiguous [80*192, 672] = [15360, 672] f32 block (flat row r = d*192 + h,
column q = w*3 + i).  Value at (r, q) = gk[(r mod 192), q] + f[(r div 192),
q mod 3].

variant 'ts3' (old baseline): 120 tiles of 128 rows; partition p of tile
t holds row 128t+p; 3 tensor_scalar/activation adds per tile (split
across DVE and ACT engines) + one contiguous 344 KB DMA store per tile.

variant 'fat' (default): G tiles (G%3==0) per DMA group; partition p of
group T holds the G consecutive rows G*p+j (j<G) of the group.  Because
128*G % 192 == 0 the h-pattern (G*p+j) % 192 is group-invariant, so a
single interleaved base table basef[p, j*672+q] = gkI[(G*p+j)%192, q]
serves every group, and d(p,T) = (128G/192)*T + (G*p)//192 makes the
f-offset a per-partition (p, i)-scalar per group -> one DVE
tensor_tensor (stride-0 broadcast) per group + one fat DMA whose
descriptors are G*2688 contiguous bytes per partition.

'ts3nc'/'fatnc': timing-only probes - same DMA structure, but slabs are
precomputed once outside the loop, so the loop is pure DMA (measures the
achievable HBM write bandwidth for that DMA granularity).  'ts3cx'/
'fatcx': compute-only probes (no out DMAs).  'fatbf': bf16 tables
(1.7e-3 rel err, no measured speedup -> not default).

HW-measured (repeat-loop slope, all 8 cores concurrent):
  - pure-DMA floor: 113.9-118.9 us depending on time window (the device
    drifts ~5% with sustained load; 41.3 MB -> 348-362 GB/s per core).
  - 'fat' g=6 bufs=8 hits the floor exactly in cool windows (113.9 us)
    and sits ~5 us above the concurrent floor in hot windows, tied with
    'ts3' (116.5 us cool).  DMA granularity (344 KB..8.25 MB), rings=2,
    bf16 tables, and bufs>8 move nothing or regress.
"""

import os
import sys

sys.path.insert(0, "/opt/trn_rl_repo")

import numpy as np

import concourse.bacc as bacc
import concourse.bass as bass
import concourse.tile as tile
from concourse import mybir
from concourse.bass_utils import run_bass_kernel_spmd

D, H, W = 160, 192, 224
B = 4
NCORES = 8
DSH = D // 2            # 80 d's per core
ROWS = DSH * H          # 15360 flat rows per core
NT = ROWS // 128        # 120 tiles of 128 rows
Q = W * 3               # 672 columns

F32 = mybir.dt.float32

# Per-tile engine choice for ts3: 'v' = VectorE tensor_scalar, 's' = ScalarE
# activation.  DVE ~531ns/tile vs ACT ~840ns/tile -> 3:2 split keeps both
# under DMA time.  (Unused by the 'fat' variant, which is all-DVE.)
VEC_FRAC_NUM = int(os.environ.get("K_VNUM", "3"))
VEC_FRAC_DEN = int(os.environ.get("K_VDEN", "5"))
SLAB_BUFS = int(os.environ.get("K_BUFS", "8"))
# HW-measured (repeat-loop slope, 8 cores): ts3 116.5us; fat g=6 bufs=8
# 113.9us = the pure-DMA (ts3nc) ceiling of ~362 GB/s/core.  g=3/g=12,
# bufs=4, rings=2 all regress.
BEST_VARIANT = os.environ.get("K_VARIANT", "fat")
# Output DMAs alternate across the two HWDGE rings (SP + ACT) when rings=2.
RINGS = int(os.environ.get("K_RINGS", "1"))
FAT_G = int(os.environ.get("K_G", "6"))

_CACHE = {}


def _build_program(
    variant: str = BEST_VARIANT,
    vnum: int = VEC_FRAC_NUM,
    vden: int = VEC_FRAC_DEN,
    bufs: int = SLAB_BUFS,
    repeat: int = 0,
    rings: int = RINGS,
    hints: bool = False,
    fat_g: int = FAT_G,
):
    """Build the SPMD program.

    variant 'ts3': 3x tensor_scalar/activation per 128-row tile, 344 KB DMAs.
    variant 'fat': fat_g tiles per group, one DVE tensor_tensor + one fat DMA.
    variant 'ts3nc'/'fatnc': pure-DMA timing probes (slabs precomputed).
    repeat > 0: timing build - output goes to internal DRAM, the whole body
      is wrapped in a For_i(repeat) loop, and a tiny dummy external output
      is written once (per-iteration time = wall-time slope between two
      repeat counts).
    """
    nc = bacc.Bacc(
        "TRN2",
        target_bir_lowering=False,
        debug=False,
        enable_asserts=False,
        num_devices=NCORES,
    )

    is_fat = variant.startswith("fat")
    is_bf = "bf" in variant              # bf16 base/ftab tables (f32 output)
    TDT = mybir.dt.bfloat16 if is_bf else F32
    nocompute = variant.endswith("nc")   # pure-DMA probe (slabs precomputed)
    nodma = variant.endswith("cx")       # pure-compute probe (no out DMAs)
    if is_fat:
        G = fat_g
        assert G % 3 == 0 and NT % G == 0
        NG = NT // G
        basef_d = nc.dram_tensor("basef", [128, G * Q], TDT, kind="ExternalInput")
        ftab_d = nc.dram_tensor("ftab", [128, NG * 3], TDT, kind="ExternalInput")
    else:
        base_d = nc.dram_tensor("base3", [3, 128, 3, W], F32, kind="ExternalInput")
        ftab_d = nc.dram_tensor("ftab", [128, NT * 3], F32, kind="ExternalInput")
    if repeat:
        out_d = nc.dram_tensor("out", [ROWS, Q], F32)  # internal scratch
        outx_d = nc.dram_tensor("outx", [128, 8], TDT, kind="ExternalOutput")
    else:
        out_d = nc.dram_tensor("out", [ROWS, Q], F32, kind="ExternalOutput")
        outx_d = None

    with tile.TileContext(nc) as tc:
        with (
            tc.tile_pool(name="consts", bufs=1) as consts,
            tc.tile_pool(name="slabs", bufs=bufs) as slabs,
        ):
            # ftab first: every tile needs it, while tile t only needs base
            # variant t%3 - loading ftab last would serialize the whole
            # input ahead of the first compute.
            if is_fat:
                ft = consts.tile([128, NG * 3], TDT, tag="ftab")
                nc.sync.dma_start(out=ft[:], in_=ftab_d[:])
                if not nocompute:
                    bf = consts.tile([128, G * Q], TDT, tag="basef")
                    nc.sync.dma_start(out=bf[:], in_=basef_d[:])
                out_r = out_d[:].rearrange("(T p j) q -> T p (j q)", p=128, j=G)

                def compute_group(dst_ap, T):
                    op2 = (
                        ft[:, T * 3 : T * 3 + 3]
                        .unsqueeze(1)
                        .broadcast_to([128, G * W, 3])
                    )
                    in0 = bf[:].rearrange("p (x c) -> p x c", c=3)
                    nc.vector.tensor_tensor(
                        out=dst_ap.rearrange("p (x c) -> p x c", c=3),
                        in0=in0,
                        in1=op2,
                        op=mybir.AluOpType.add,
                    )

                if nocompute:
                    fixed = []
                    for i in range(bufs):
                        s = consts.tile([128, G * Q], F32, tag=f"s{i}")
                        nc.vector.memset(s[:], float(i + 1))
                        fixed.append(s)

                last_slab = []

                def body(_iv=None):
                    for T in range(NG):
                        deng = [nc.sync, nc.scalar][T % min(rings, 2)]
                        if nocompute:
                            deng.dma_start(out=out_r[T], in_=fixed[T % bufs][:])
                        else:
                            slab = slabs.tile([128, G * Q], F32, tag="slab")
                            compute_group(slab[:], T)
                            if nodma:
                                last_slab[:] = [slab[:, 0:8]]
                            else:
                                deng.dma_start(out=out_r[T], in_=slab[:])

            else:
                ft = consts.tile([128, NT * 3], F32, tag="ftab")
                nc.sync.dma_start(out=ft[:], in_=ftab_d[:])
                base_t = []
                for v in range(3):
                    bt = consts.tile([128, 3, W], F32, tag=f"base{v}")
                    nc.sync.dma_start(out=bt[:], in_=base_d[v])
                    base_t.append(bt)

                def compute_tile(dst, t, use_vec):
                    bt = base_t[t % 3]
                    for i in range(3):
                        sc = ft[:, t * 3 + i : t * 3 + i + 1]
                        if use_vec:
                            nc.vector.tensor_scalar_add(dst[:, :, i], bt[:, i, :], sc)
                        else:
                            nc.scalar.activation(
                                dst[:, :, i],
                                bt[:, i, :],
                                mybir.ActivationFunctionType.Identity,
                                bias=sc,
                                scale=1.0,
                            )

                if nocompute:
                    fixed = []
                    for i in range(bufs):
                        s = consts.tile([128, W, 3], F32, tag=f"s{i}")
                        compute_tile(s, i, True)
                        fixed.append(s)

                last_slab = []

                def body(_iv=None):
                    for t in range(NT):
                        deng = [nc.sync, nc.scalar, nc.gpsimd][t % rings]
                        if nocompute:
                            if variant == "ts3wnc":
                                # independent compute stream: same op mix as
                                # the real kernel, but the DMAs below do NOT
                                # depend on it (probes SBUF/power interference
                                # vs dependency overhead).
                                scr = slabs.tile([128, W, 3], F32, tag="scr")
                                compute_tile(scr, t, (t * vnum) % vden < vnum)
                            deng.dma_start(
                                out=out_d[bass.ts(t, 128), :],
                                in_=fixed[t % bufs][:].rearrange("p w i -> p (w i)"),
                            )
                            continue
                        slab = slabs.tile([128, W, 3], F32, tag="slab")
                        use_vec = (t * vnum) % vden < vnum
                        compute_tile(slab, t, use_vec)
                        if nodma:
                            last_slab[:] = [
                                slab[:].rearrange("p w i -> p (w i)")[:, 0:8]
                            ]
                        else:
                            deng.dma_start(
                                out=out_d[bass.ts(t, 128), :],
                                in_=slab[:].rearrange("p w i -> p (w i)"),
                            )

            if repeat:
                he = (
                    (
                        mybir.EngineType.SP,
                        mybir.EngineType.Activation,
                        mybir.EngineType.DVE,
                    )
                    if hints
                    else ()
                )
                with tc.For_i(0, repeat, 1, hint_engines=he) as _i:
                    body(_i)
                src = last_slab[0] if (nodma and last_slab) else ft[:, 0:8]
                nc.sync.dma_start(out=outx_d[:], in_=src)
            else:
                body()

    nc.compile()
    return nc


def _fgk(matrix: np.ndarray, c: int):
    """Exact-math per-core f [80,3], g [192,3], k [224,3] tables."""
    b, dlo = c // 2, DSH * (c % 2)
    M = matrix[b].astype(np.float64)
    A = M[:, :3] - np.eye(3)
    tvec = M[:, 3]
    dm = np.arange(dlo, dlo + DSH) - (D - 1) / 2.0
    hm = np.arange(H) - (H - 1) / 2.0
    wm = np.arange(W) - (W - 1) / 2.0
    f = dm[:, None] * A[:, 0][None, :] + tvec[None, :]      # [80, 3]
    g = hm[:, None] * A[:, 1][None, :]                      # [192, 3]
    k = wm[:, None] * A[:, 2][None, :]                      # [224, 3]
    return f.astype(np.float32), g, k


def _host_inputs(
    matrix: np.ndarray, variant: str = "ts3", fat_g: int = FAT_G
) -> list[dict[str, np.ndarray]]:
    """Per-core input maps.  Core c: batch c//2, d-range [80*(c%2), +80)."""
    in_maps = []
    is_fat = variant.startswith("fat")
    for c in range(NCORES):
        f32, g, k = _fgk(matrix, c)
        p = np.arange(128)
        if is_fat:
            npdt = mybir.dt.np(mybir.dt.bfloat16) if "bf" in variant else np.float32
            G = fat_g
            NG = NT // G
            gkI = (g[:, None, :] + k[None, :, :]).reshape(H, Q)  # [192, 672]
            hrow = (G * p[:, None] + np.arange(G)[None, :]) % H  # [128, G]
            basef = gkI[hrow].reshape(128, G * Q)
            dl = (128 * G * np.arange(NG)[None, :]) // H + (G * p[:, None]) // H
            ftab = f32[dl].reshape(128, NG * 3)
            in_maps.append(
                {
                    "basef": np.ascontiguousarray(basef.astype(npdt)),
                    "ftab": np.ascontiguousarray(ftab.astype(npdt)),
                }
            )
        else:
            gk = (g[:, :, None] + k.T[None, :, :]).astype(np.float32)  # [192,3,224]
            gk_row = gk.reshape(H, 3 * W)  # planar (i, w)
            base = np.tile(gk_row, (2, 1))[: 3 * 128].reshape(3, 128, 3, W)
            r = 128 * np.arange(NT)[None, :] + p[:, None]  # [128, NT]
            ftab = f32[r // H].reshape(128, NT * 3)
            in_maps.append(
                {
                    "base3": np.ascontiguousarray(base, np.float32),
                    "ftab": np.ascontiguousarray(ftab, np.float32),
                }
            )
    return in_maps


def _run(matrix: np.ndarray, trace: bool = False, tmpdir=None, **build_kw):
    key = tuple(sorted(build_kw.items()))
    if key not in _CACHE:
        _CACHE[key] = _build_program(**build_kw)
    nc = _CACHE[key]
    res = run_bass_kernel_spmd(
        nc,
        _host_inputs(
            matrix,
            build_kw.get("variant", BEST_VARIANT),
            build_kw.get("fat_g", FAT_G),
        ),
        list(range(NCORES)),
        trace=trace,
        tmpdir=tmpdir,
    )
    if build_kw.get("repeat"):
        return None, res
    out = np.empty((B, D, H, W, 3), np.float32)
    for c in range(NCORES):
        b, dlo = c // 2, DSH * (c % 2)
        out[b, dlo : dlo + DSH] = res.results[c]["out"].reshape(DSH, H, W, 3)
    return out, res


def kernel(matrix: np.ndarray) -> np.ndarray:
    out, _ = _run(np.asarray(matrix))
    return out


# revision 23
# speedup vs baseline: 1.0071x; 1.0071x over previous
"""AffineToDenseShift Trainium2 kernel.

Computes out[b,d,h,w,i] = ((A_b - I) @ mesh(d,h,w) + t_b)[i] for the
centered ij meshgrid of shape (160, 192, 224), batch 4, f32.

The field is additively separable: out = f_i(d) + g_i(h) + k_i(w) with
f_i(d) = M[i,0]*(d-cD) + t[i], g_i(h) = M[i,1]*(h-cH), k_i(w) = M[i,2]*(w-cW),
M = A - I.  Inputs are tiny (48 floats/batch); the problem is purely about
materializing and writing the 330 MB output at HBM line rate (~358 GB/s
per NeuronCore -> 115.3 us floor for the 41.3 MB per-core slice).

Sharding: 8 cores = 4 batches x 2 halves of D.  Each core writes a flat
contiguous [80*192, 672] = [15360, 672] f32 block (flat row r = d*192 + h,
column q = w*3 + i).  Value at (r, q) = gk[(r mod 192), q] + f[(r div 192),
q mod 3].

variant 'ts3' (old baseline): 120 tiles of 128 rows; partition p of tile
t holds row 128t+p; 3 tensor_scalar/activation adds per tile (split
across DVE and ACT engines) + one contiguous 344 KB DMA store per tile.

variant 'fat' (default): G tiles (G%3==0) per DMA group; partition p of
group T holds the G consecutive rows G*p+j (j<G) of the group.  Because
128*G % 192 == 0 the h-pattern (G*p+j) % 192 is group-invariant, so a
single interleaved base table basef[p, j*672+q] = gkI[(G*p+j)%192, q]
serves every group, and d(p,T) = (128G/192)*T + (G*p)//192 makes the
f-offset a per-partition (p, i)-scalar per group -> per-group compute +
one fat DMA whose descriptors are G*2688 contiguous bytes per partition.
'fat' computes each group with one DVE tensor_tensor (stride-0
broadcast); 'fat3' (default) uses 3 strided tensor_scalar ops instead
(162 vs 123 G elem/s measured -> lower DVE duty, ~4 us/iter faster
in interleaved A/B because compute interferes less with the DMA
stream).

'ts3nc'/'fatnc': timing-only probes - same DMA structure, but slabs are
precomputed once outside the loop, so the loop is pure DMA (measures the
achievable HBM write bandwidth for that DMA granularity).  'ts3cx'/
'fatcx': compute-only probes (no out DMAs).  'fatbf': bf16 tables
(1.7e-3 rel err, no measured speedup -> not default).

HW-measured (repeat-loop slope, all 8 cores concurrent):
  - pure-DMA floor: 113.9-118.9 us depending on time window (the device
    drifts ~5% with sustained load; 41.3 MB -> 348-362 GB/s per core).
  - 'fat' g=6 bufs=8 hits the floor exactly in cool windows (113.9 us)
    and sits ~5 us above the concurrent floor in hot windows, tied with
    'ts3' (116.5 us cool).  DMA granularity (344 KB..8.25 MB), rings=2,
    bf16 tables, and bufs>8 move nothing or regress.
"""

import os
import sys

sys.path.insert(0, "/opt/trn_rl_repo")

import numpy as np

import concourse.bacc as bacc
import concourse.bass as bass
import concourse.tile as tile
from concourse import mybir
from concourse.bass_utils import run_bass_kernel_spmd

D, H, W = 160, 192, 224
B = 4
NCORES = 8
DSH = D // 2            # 80 d's per core
ROWS = DSH * H          # 15360 flat rows per core
NT = ROWS // 128        # 120 tiles of 128 rows
Q = W * 3               # 672 columns

F32 = mybir.dt.float32

# Per-group/tile engine choice: 'v' = VectorE tensor_scalar, 's' = ScalarE
# activation.  Default 1/1 = all-DVE (fat3's DVE duty is only ~56% so no
# ACT assist is needed; ts3's tuned split was 3/5).
VEC_FRAC_NUM = int(os.environ.get("K_VNUM", "1"))
VEC_FRAC_DEN = int(os.environ.get("K_VDEN", "1"))
SLAB_BUFS = int(os.environ.get("K_BUFS", "8"))
# HW-measured (repeat-loop slope, 8 cores): ts3 116.5us; fat g=6 bufs=8
# 113.9us (cool window).  Interleaved same-window: fat3 119.9 vs fat 124.1
# vs pure-DMA floor 112.9 -- the 3-op tensor_scalar form (162 G elem/s vs
# 123 for broadcast tensor_tensor) cuts compute duty and its interference
# with the DMA stream.  g=3/g=12, bufs=4, rings=2, bf16 all regress/no-op.
BEST_VARIANT = os.environ.get("K_VARIANT", "fat3")
# Output DMAs alternate across the two HWDGE rings (SP + ACT) when rings=2.
RINGS = int(os.environ.get("K_RINGS", "1"))
FAT_G = int(os.environ.get("K_G", "6"))

_CACHE = {}


def _build_program(
    variant: str = BEST_VARIANT,
    vnum: int = VEC_FRAC_NUM,
    vden: int = VEC_FRAC_DEN,
    bufs: int = SLAB_BUFS,
    repeat: int = 0,
    rings: int = RINGS,
    hints: bool = False,
    fat_g: int = FAT_G,
):
    """Build the SPMD program.

    variant 'ts3': 3x tensor_scalar/activation per 128-row tile, 344 KB DMAs.
    variant 'fat': fat_g tiles per group, one DVE tensor_tensor + one fat DMA.
    variant 'ts3nc'/'fatnc': pure-DMA timing probes (slabs precomputed).
    repeat > 0: timing build - output goes to internal DRAM, the whole body
      is wrapped in a For_i(repeat) loop, and a tiny dummy external output
      is written once (per-iteration time = wall-time slope between two
      repeat counts).
    """
    nc = bacc.Bacc(
        "TRN2",
        target_bir_lowering=False,
        debug=False,
        enable_asserts=False,
        num_devices=NCORES,
    )

    is_fat = variant.startswith("fat")
    is_bf = "bf" in variant              # bf16 base/ftab tables (f32 output)
    TDT = mybir.dt.bfloat16 if is_bf else F32
    nocompute = variant.endswith("nc")   # pure-DMA probe (slabs precomputed)
    nodma = variant.endswith("cx")       # pure-compute probe (no out DMAs)
    if is_fat:
        G = fat_g
        assert G % 3 == 0 and NT % G == 0
        NG = NT // G
        basef_d = nc.dram_tensor("basef", [128, G * Q], TDT, kind="ExternalInput")
        ftab_d = nc.dram_tensor("ftab", [128, NG * 3], TDT, kind="ExternalInput")
    else:
        base_d = nc.dram_tensor("base3", [3, 128, 3, W], F32, kind="ExternalInput")
        ftab_d = nc.dram_tensor("ftab", [128, NT * 3], F32, kind="ExternalInput")
    if repeat:
        out_d = nc.dram_tensor("out", [ROWS, Q], F32)  # internal scratch
        outx_d = nc.dram_tensor("outx", [128, 8], TDT, kind="ExternalOutput")
    else:
        out_d = nc.dram_tensor("out", [ROWS, Q], F32, kind="ExternalOutput")
        outx_d = None

    with tile.TileContext(nc) as tc:
        with (
            tc.tile_pool(name="consts", bufs=1) as consts,
            tc.tile_pool(name="slabs", bufs=bufs) as slabs,
        ):
            # ftab first: every tile needs it, while tile t only needs base
            # variant t%3 - loading ftab last would serialize the whole
            # input ahead of the first compute.
            if is_fat:
                ft = consts.tile([128, NG * 3], TDT, tag="ftab")
                nc.sync.dma_start(out=ft[:], in_=ftab_d[:])
                if not nocompute:
                    bf = consts.tile([128, G * Q], TDT, tag="basef")
                    nc.sync.dma_start(out=bf[:], in_=basef_d[:])
                out_r = out_d[:].rearrange("(T p j) q -> T p (j q)", p=128, j=G)

                def compute_group(dst_ap, T, use_vec=True):
                    if variant.startswith("fat3"):
                        # 3 strided tensor_scalar/activation ops: HW-measured
                        # 162 G elem/s vs 123 for the broadcast tensor_tensor
                        # -> lower engine duty, optional DVE/ACT group split.
                        dst3 = dst_ap.rearrange("p (x c) -> p x c", c=3)
                        in3 = bf[:].rearrange("p (x c) -> p x c", c=3)
                        for i in range(3):
                            sc = ft[:, T * 3 + i : T * 3 + i + 1]
                            if use_vec:
                                nc.vector.tensor_scalar_add(
                                    dst3[:, :, i], in3[:, :, i], sc
                                )
                            else:
                                nc.scalar.activation(
                                    dst3[:, :, i],
                                    in3[:, :, i],
                                    mybir.ActivationFunctionType.Identity,
                                    bias=sc,
                                    scale=1.0,
                                )
                        return
                    op2 = (
                        ft[:, T * 3 : T * 3 + 3]
                        .unsqueeze(1)
                        .broadcast_to([128, G * W, 3])
                    )
                    in0 = bf[:].rearrange("p (x c) -> p x c", c=3)
                    nc.vector.tensor_tensor(
                        out=dst_ap.rearrange("p (x c) -> p x c", c=3),
                        in0=in0,
                        in1=op2,
                        op=mybir.AluOpType.add,
                    )

                if nocompute:
                    fixed = []
                    for i in range(bufs):
                        s = consts.tile([128, G * Q], F32, tag=f"s{i}")
                        nc.vector.memset(s[:], float(i + 1))
                        fixed.append(s)

                last_slab = []

                def body(_iv=None):
                    for T in range(NG):
                        deng = [nc.sync, nc.scalar][T % min(rings, 2)]
                        if nocompute:
                            deng.dma_start(out=out_r[T], in_=fixed[T % bufs][:])
                        else:
                            slab = slabs.tile([128, G * Q], F32, tag="slab")
                            compute_group(slab[:], T, (T * vnum) % vden < vnum)
                            if nodma:
                                last_slab[:] = [slab[:, 0:8]]
                            else:
                                deng.dma_start(out=out_r[T], in_=slab[:])

            else:
                ft = consts.tile([128, NT * 3], F32, tag="ftab")
                nc.sync.dma_start(out=ft[:], in_=ftab_d[:])
                base_t = []
                for v in range(3):
                    bt = consts.tile([128, 3, W], F32, tag=f"base{v}")
                    nc.sync.dma_start(out=bt[:], in_=base_d[v])
                    base_t.append(bt)

                def compute_tile(dst, t, use_vec):
                    bt = base_t[t % 3]
                    for i in range(3):
                        sc = ft[:, t * 3 + i : t * 3 + i + 1]
                        if use_vec:
                            nc.vector.tensor_scalar_add(dst[:, :, i], bt[:, i, :], sc)
                        else:
                            nc.scalar.activation(
                                dst[:, :, i],
                                bt[:, i, :],
                                mybir.ActivationFunctionType.Identity,
                                bias=sc,
                                scale=1.0,
                            )

                if nocompute:
                    fixed = []
                    for i in range(bufs):
                        s = consts.tile([128, W, 3], F32, tag=f"s{i}")
                        compute_tile(s, i, True)
                        fixed.append(s)

                last_slab = []

                def body(_iv=None):
                    for t in range(NT):
                        deng = [nc.sync, nc.scalar, nc.gpsimd][t % rings]
                        if nocompute:
                            if variant == "ts3wnc":
                                # independent compute stream: same op mix as
                                # the real kernel, but the DMAs below do NOT
                                # depend on it (probes SBUF/power interference
                                # vs dependency overhead).
                                scr = slabs.tile([128, W, 3], F32, tag="scr")
                                compute_tile(scr, t, (t * vnum) % vden < vnum)
                            deng.dma_start(
                                out=out_d[bass.ts(t, 128), :],
                                in_=fixed[t % bufs][:].rearrange("p w i -> p (w i)"),
                            )
                            continue
                        slab = slabs.tile([128, W, 3], F32, tag="slab")
                        use_vec = (t * vnum) % vden < vnum
                        compute_tile(slab, t, use_vec)
                        if nodma:
                            last_slab[:] = [
                                slab[:].rearrange("p w i -> p (w i)")[:, 0:8]
                            ]
                        else:
                            deng.dma_start(
                                out=out_d[bass.ts(t, 128), :],
                                in_=slab[:].rearrange("p w i -> p (w i)"),
                            )

            if repeat:
                he = (
                    (
                        mybir.EngineType.SP,
                        mybir.EngineType.Activation,
                        mybir.EngineType.DVE,
                    )
                    if hints
                    else ()
                )
                with tc.For_i(0, repeat, 1, hint_engines=he) as _i:
                    body(_i)
                src = last_slab[0] if (nodma and last_slab) else ft[:, 0:8]
                nc.sync.dma_start(out=outx_d[:], in_=src)
            else:
                body()

    nc.compile()
    return nc


def _fgk(matrix: np.ndarray, c: int):
    """Exact-math per-core f [80,3], g [192,3], k [224,3] tables."""
    b, dlo = c // 2, DSH * (c % 2)
    M = matrix[b].astype(np.float64)
    A = M[:, :3] - np.eye(3)
    tvec = M[:, 3]
    dm = np.arange(dlo, dlo + DSH) - (D - 1) / 2.0
    hm = np.arange(H) - (H - 1) / 2.0
    wm = np.arange(W) - (W - 1) / 2.0
    f = dm[:, None] * A[:, 0][None, :] + tvec[None, :]      # [80, 3]
    g = hm[:, None] * A[:, 1][None, :]                      # [192, 3]
    k = wm[:, None] * A[:, 2][None, :]                      # [224, 3]
    return f.astype(np.float32), g, k


def _host_inputs(
    matrix: np.ndarray, variant: str = "ts3", fat_g: int = FAT_G
) -> list[dict[str, np.ndarray]]:
    """Per-core input maps.  Core c: batch c//2, d-range [80*(c%2), +80)."""
    in_maps = []
    is_fat = variant.startswith("fat")
    for c in range(NCORES):
        f32, g, k = _fgk(matrix, c)
        p = np.arange(128)
        if is_fat:
            npdt = mybir.dt.np(mybir.dt.bfloat16) if "bf" in variant else np.float32
            G = fat_g
            NG = NT // G
            gkI = (g[:, None, :] + k[None, :, :]).reshape(H, Q)  # [192, 672]
            hrow = (G * p[:, None] + np.arange(G)[None, :]) % H  # [128, G]
            basef = gkI[hrow].reshape(128, G * Q)
            dl = (128 * G * np.arange(NG)[None, :]) // H + (G * p[:, None]) // H
            ftab = f32[dl].reshape(128, NG * 3)
            in_maps.append(
                {
                    "basef": np.ascontiguousarray(basef.astype(npdt)),
                    "ftab": np.ascontiguousarray(ftab.astype(npdt)),
                }
            )
        else:
            gk = (g[:, :, None] + k.T[None, :, :]).astype(np.float32)  # [192,3,224]
            gk_row = gk.reshape(H, 3 * W)  # planar (i, w)
            base = np.tile(gk_row, (2, 1))[: 3 * 128].reshape(3, 128, 3, W)
            r = 128 * np.arange(NT)[None, :] + p[:, None]  # [128, NT]
            ftab = f32[r // H].reshape(128, NT * 3)
            in_maps.append(
                {
                    "base3": np.ascontiguousarray(base, np.float32),
                    "ftab": np.ascontiguousarray(ftab, np.float32),
                }
            )
    return in_maps


def _run(matrix: np.ndarray, trace: bool = False, tmpdir=None, **build_kw):
    key = tuple(sorted(build_kw.items()))
    if key not in _CACHE:
        _CACHE[key] = _build_program(**build_kw)
    nc = _CACHE[key]
    res = run_bass_kernel_spmd(
        nc,
        _host_inputs(
            matrix,
            build_kw.get("variant", BEST_VARIANT),
            build_kw.get("fat_g", FAT_G),
        ),
        list(range(NCORES)),
        trace=trace,
        tmpdir=tmpdir,
    )
    if build_kw.get("repeat"):
        return None, res
    out = np.empty((B, D, H, W, 3), np.float32)
    for c in range(NCORES):
        b, dlo = c // 2, DSH * (c % 2)
        out[b, dlo : dlo + DSH] = res.results[c]["out"].reshape(DSH, H, W, 3)
    return out, res


def kernel(matrix: np.ndarray) -> np.ndarray:
    out, _ = _run(np.asarray(matrix))
    return out
